# revision 77
# baseline (speedup 1.0000x reference)
"""Multi-head attention (B=4, N=2048, C=1024, H=16, D=64) on 8 TRN2 NeuronCores.

Sharding: data-parallel over batch (4) x tensor-parallel over heads (2 groups
of 8 heads).  Device d handles batch d//2 and head-group d%2.  Host sums the
two proj partials per batch and transposes back.

Kernel structure: one flat software pipeline over 256 (head-pair, q-chunk,
key-chunk) slots.  Slot g emits S(g) on the tensor engine + exp(g) on the
scalar engine; PV for head 0 runs at lag 2 and head 1 at lag 3 (the stagger
lets the PSUM->SBUF copy of the finished O accumulator overlap the next
chunk's first PV without double-buffering PSUM).  All remaining tensor-engine
work (V pass, q/k projections for later head pairs, softmax-denominator
reciprocal broadcast, output projection) drains from a slot-scheduled pending
queue so the PE stream never idles.  The S/exp stream runs up to ~16 slots
ahead of PV, banking exp results in a deep e-tile ring during the PE-heavy
first chunk so the scalar engine's exp throughput never paces the pipeline.
Matmul operands are bf16 (same PE cost in fp32-accumulate matmuls, half the
DMA/SBUF footprint); PSUM accumulation, softmax denominators and reciprocals
stay fp32.  A short warm-up of throwaway matmuls holds the PE busy from t=0
so the p-state ramp finishes before real work.
"""

import os
import sys

for _p in ("/opt/trn_rl_repo", "/root/.axon_site/_ro/trn_rl_repo"):
    if os.path.isdir(_p) and _p not in sys.path:
        sys.path.insert(0, _p)

import numpy as np

B, N, C = 4, 2048, 1024
H_LOC = 8  # heads per device
D = 64
CH = 512  # qkv channels per device (H_LOC * D)
P = 128
SCALE = 0.125  # D ** -0.5
NKC = N // P  # 16 key chunks
NQC = N // 512  # 4 query chunks of 512
NCI = C // P  # 8 c_in chunks
NPAIR = 4  # head pairs per device
NCHUNK = NPAIR * NQC  # 16
NSLOT = NCHUNK * NKC  # 256

_CACHE = {}
LAST_EXEC_TIME_NS = None


def _build():
    import concourse.bacc as bacc
    import concourse.mybir as mybir
    import concourse.tile as tile

    F32 = mybir.dt.float32
    F32R = mybir.dt.float32r
    BF16 = mybir.dt.bfloat16
    Exp = mybir.ActivationFunctionType.Exp

    nc = bacc.Bacc("TRN2", target_bir_lowering=False, debug=False)

    xT_d = nc.dram_tensor("xT", [C, N], BF16, kind="ExternalInput")
    wq_d = nc.dram_tensor("wq", [C, CH], BF16, kind="ExternalInput")
    wk_d = nc.dram_tensor("wk", [C, CH], BF16, kind="ExternalInput")
    wv_d = nc.dram_tensor("wv", [C, CH], BF16, kind="ExternalInput")
    pw_d = nc.dram_tensor("pw", [CH, C], BF16, kind="ExternalInput")
    bias_d = nc.dram_tensor("bias", [C], F32, kind="ExternalInput")
    yT_d = nc.dram_tensor("yT", [C, N], BF16, kind="ExternalOutput")

    xT_re = xT_d[:].rearrange("(c p) n -> p c n", p=P)
    wq_re = wq_d[:].rearrange("(c p) m -> p c m", p=P)
    wk_re = wk_d[:].rearrange("(c p) m -> p c m", p=P)
    wv_re = wv_d[:].rearrange("(c p) m -> p c m", p=P)
    pw_re = pw_d[:].rearrange("(c p) m -> p c m", p=P)
    bias_re = bias_d[:].rearrange("(a p) -> p a", p=P)
    yT_re = yT_d[:].rearrange("(a p) n -> p a n", p=P)

    from collections import defaultdict, deque
    from contextlib import ExitStack

    with tile.TileContext(nc) as tc, ExitStack() as st:
        ps_s = st.enter_context(tc.tile_pool(name="ps_s", bufs=2, space="PSUM"))
        ps_o = st.enter_context(tc.tile_pool(name="ps_o", bufs=1, space="PSUM"))
        ps_mm = st.enter_context(tc.tile_pool(name="ps_mm", bufs=2, space="PSUM"))
        sb = st.enter_context(tc.tile_pool(name="sb", bufs=1))

        # ---- persistent SBUF tiles ----
        xsb = sb.tile([P, NCI, N], BF16)
        wv_sb = sb.tile([P, NCI, CH], BF16)
        wq01 = sb.tile([P, NCI, 2 * P], BF16)
        wk01 = sb.tile([P, NCI, 2 * P], BF16)
        wq23 = sb.tile([P, NCI, 2 * P], BF16)
        wk23 = sb.tile([P, NCI, 2 * P], BF16)
        v_sb = sb.tile([P, NKC, H_LOC * 65], BF16)
        ones_src = sb.tile([P, NKC, H_LOC], F32)
        oc = sb.tile([1, D], F32R)
        ocH = sb.tile([65, D], F32R)  # row 64 = ones, same partition as denom
        dmy = sb.tile([1, 512], F32R)
        ot = [
            sb.tile([P, N], BF16, name=f"ot{t}", tag=f"ot{t}")
            for t in range(NPAIR)
        ]
        pw_sb = sb.tile([P, NPAIR, C], BF16)
        bias_sb = sb.tile([P, NCI], F32)

        # ---- constants (dummy-matmul inputs first so warm-up starts asap) ----
        nc.vector.memset(dmy.bitcast(F32), 1.0)
        nc.vector.memset(oc.bitcast(F32), 1.0)
        nc.vector.memset(ocH.bitcast(F32), 1.0)
        nc.vector.memset(ones_src, 1.0)
        nc.vector.tensor_copy(
            v_sb.rearrange("p k (h e) -> p k h e", e=65)[:, :, :, 64:65],
            ones_src.rearrange("p k (h o) -> p k h o", o=1),
        )

        # ---- PE warm-up: hold the engine busy through the p-state ramp ----
        n_wrm = [0]

        def warm(k):
            for _ in range(k):
                wrm = ps_o.tile(
                    [D, 512], F32, name=f"wrm{n_wrm[0]}", tag="o"
                )
                n_wrm[0] += 1
                nc.tensor.matmul(wrm, lhsT=oc, rhs=dmy, start=True, stop=True)

        warm(15)

        # ---- startup DMAs (weights on sync queue, x stream on gpsimd) ----
        # first weight/x loads split in c-halves: the first half of the
        # first q piece can start ~1.5us earlier
        nc.sync.dma_start(out=wq01[:, 0:4, :], in_=wq_re[:, 0:4, 0 : 2 * P])
        nc.gpsimd.dma_start(out=xsb[:, 0:4, 0:256], in_=xT_re[:, 0:4, 0:256])
        nc.sync.dma_start(out=wq01[:, 4:8, :], in_=wq_re[:, 4:8, 0 : 2 * P])
        nc.gpsimd.dma_start(out=xsb[:, 4:8, 0:256], in_=xT_re[:, 4:8, 0:256])
        # tiny shim: delays wk01's HWDGE slot just enough that the first x
        # slice wins the DMA-engine grant race (and loads bias early)
        nc.sync.dma_start(out=bias_sb, in_=bias_re)
        nc.sync.dma_start(out=wk01, in_=wk_re[:, :, 0 : 2 * P])
        nc.gpsimd.dma_start(out=xsb[:, :, 256:512], in_=xT_re[:, :, 256:512])
        nc.sync.dma_start(out=wv_sb[:, 0:4, :], in_=wv_re[:, 0:4, :])
        nc.sync.dma_start(out=wv_sb[:, 4:8, :], in_=wv_re[:, 4:8, :])
        for nb in range(1, NQC):
            sl = slice(nb * 512, (nb + 1) * 512)
            nc.gpsimd.dma_start(out=xsb[:, :, sl], in_=xT_re[:, :, sl])

        # ---- helpers ----
        qk_tiles = {}  # t -> (qT, kT)

        def alloc_qk(t):
            qT_t = sb.tile([P, N], BF16, name=f"qT{t}", tag="qT", bufs=2)
            kT_t = sb.tile([P, N], BF16, name=f"kT{t}", tag="kT", bufs=2)
            qk_tiles[t] = (qT_t, kT_t)

        def w_src(t, kind):
            w = (wq01 if kind == "q" else wk01) if t < 2 else (
                wq23 if kind == "q" else wk23
            )
            csl = slice((t % 2) * P, (t % 2) * P + P)
            return w, csl

        piece_ready = set()  # (t, kind, nb) -> q/k projection piece emitted

        def emit_qk_piece(t, kind, col0, w, c0=0, c1=NCI, ps_cell=None):
            """Partial q/k projection: rows of qT/kT[t], cols [col0, col0+w)."""
            w_sb, csl = w_src(t, kind)
            dst = qk_tiles[t][0 if kind == "q" else 1]
            if ps_cell is None:
                ps_cell = {}
            if c0 == 0:
                ps_cell["ps"] = ps_mm.tile(
                    [P, w], F32, name=f"qk{t}{kind}{col0}", tag="mm"
                )
            ps = ps_cell["ps"]
            nsl = slice(col0, col0 + w)
            for c in range(c0, c1):
                nc.tensor.matmul(
                    ps,
                    lhsT=w_sb[:, c, csl],
                    rhs=xsb[:, c, nsl],
                    start=(c == 0),
                    stop=(c == NCI - 1),
                )
            if c1 == NCI:
                nc.vector.tensor_copy(dst[:, nsl], ps)
                if (col0 + w) % 512 == 0:
                    piece_ready.add((t, kind, (col0 + w - 1) // 512))
            return ps_cell

        def emit_v(kc, c0=0, c1=NCI, cell=None):
            if cell is None:
                cell = {}
            if c0 == 0:
                cell["ps"] = ps_mm.tile(
                    [P, CH], F32, name=f"v{kc}", tag="mm"
                )
            v_ps = cell["ps"]
            ksl = slice(kc * P, (kc + 1) * P)
            for c in range(c0, c1):
                nc.tensor.matmul(
                    v_ps,
                    lhsT=xsb[:, c, ksl],
                    rhs=wv_sb[:, c, :],
                    start=(c == 0),
                    stop=(c == NCI - 1),
                )
            if c1 < NCI:
                return cell
            nc.vector.tensor_copy(
                v_sb.rearrange("p k (h e) -> p k h e", e=65)[:, kc, :, 0:64],
                v_ps.rearrange("p (h e) -> p h e", e=64),
            )

        e_tiles = {}
        E_BUFS = 20
        s_next = [0]

        def emit_s(g):
            i, kc = divmod(g, NKC)
            t, qc = divmod(i, NQC)
            qT_t, kT_t = qk_tiles[t]
            ksl = slice(kc * P, (kc + 1) * P)
            qsl = slice(qc * 512, (qc + 1) * 512)
            s = ps_s.tile([P, 1024], F32, name=f"s{g}", tag="s")
            nc.tensor.matmul(
                s[:, 0:512], lhsT=kT_t[0:D, ksl], rhs=qT_t[0:D, qsl],
                start=True, stop=True,
            )
            nc.tensor.matmul(
                s[:, 512:1024], lhsT=kT_t[D:P, ksl], rhs=qT_t[D:P, qsl],
                start=True, stop=True,
            )
            e = sb.tile([P, 1024], BF16, name=f"e{g}", tag="e", bufs=E_BUFS)
            nc.scalar.activation(e, s, Exp, scale=SCALE)
            e_tiles[g] = e

        def s_deps_ready(j):
            i, kc = divmod(j, NKC)
            t, qc = divmod(i, NQC)
            return (t, "q", qc) in piece_ready and (
                t, "k", kc // 4
            ) in piece_ready

        def emit_s_upto(g, budget=2):
            """Emit S/exp as far ahead of the PV stream as the e-tile ring
            and the q/k piece availability allow -- banking exps during
            PE-heavy phases so the scalar engine never paces the pipeline."""
            while (
                budget > 0
                and s_next[0] < NSLOT
                and s_next[0] <= g + E_BUFS - 4
                and s_deps_ready(s_next[0])
            ):
                emit_s(s_next[0])
                s_next[0] += 1
                budget -= 1

        o_ps_tiles = {}
        osb_tiles = {}

        def emit_pv(j, hh):
            i, pv = divmod(j, NKC)
            t = i // NQC
            h = 2 * t + hh
            if pv == 0 and hh == 0:
                o_ps_tiles[i] = ps_o.tile(
                    [65, 1024], F32, name=f"o{i}", tag="o"
                )
            o_ps = o_ps_tiles[i]
            e = e_tiles[j]
            nc.tensor.matmul(
                o_ps[:, 512 * hh : 512 * hh + 512],
                lhsT=v_sb[:, pv, 65 * h : 65 * h + 65],
                rhs=e[:, 512 * hh : 512 * hh + 512],
                start=(pv == 0),
                stop=(pv == NKC - 1),
            )
            if hh == 1:
                del e_tiles[j]
            if pv == NKC - 1:

                def copy_out():
                    osb = sb.tile(
                        [65, 512], F32R, name=f"osb{i}_{hh}",
                        tag=f"osb{hh}", bufs=2,
                    )
                    if i == NCHUNK - 1 and hh == 1:
                        # tail: scalar engine is idle; parallel O copies
                        nc.scalar.copy(
                            osb, o_ps[:, 512 * hh : 512 * hh + 512]
                        )
                    else:
                        nc.vector.tensor_copy(
                            osb, o_ps[:, 512 * hh : 512 * hh + 512]
                        )
                    osb_tiles[(i, hh)] = osb

                return copy_out
            return None

        def norm_fillers(i):
            t, qc = divmod(i, NQC)
            qsl = slice(qc * 512, (qc + 1) * 512)
            out = []
            rb_cell = {}

            def recip_rb(hh):
                def f():
                    osb = osb_tiles[(i, hh)]
                    with nc.allow_low_precision(
                        reason="f32r is bit-identical to f32 here"
                    ):
                        nc.vector.reciprocal(osb[64:65, :], osb[64:65, :])
                    rb = ps_mm.tile(
                        [D, 512], F32, name=f"rb{i}_{hh}", tag="mm"
                    )
                    nc.tensor.matmul(
                        rb, lhsT=ocH[64:65, :], rhs=osb[64:65, :],
                        start=True, stop=True,
                    )
                    rb_cell[hh] = rb

                return f

            def mult(hh):
                def f():
                    osb = osb_tiles.pop((i, hh))
                    nc.vector.tensor_mul(
                        ot[t][64 * hh : 64 * hh + 64, qsl],
                        osb[0:64, :],
                        rb_cell.pop(hh),
                    )

                return f

            return [recip_rb(0), recip_rb(1), mult(0), mult(1)]

        def qk_lo_pair(t, nb):
            cells = {}

            def f():
                if t not in qk_tiles:
                    alloc_qk(t)
                for kind in ("q", "k"):
                    cells[kind] = emit_qk_piece(
                        t, kind, nb * 512, 512, c0=0, c1=4
                    )

            return f, cells

        def qk_hi(t, nb, kind, cells):
            def f():
                emit_qk_piece(
                    t, kind, nb * 512, 512, c0=4, c1=NCI,
                    ps_cell=cells[kind],
                )

            return f

        acc_tiles = {}

        def partial_piece(co):
            """ci 0..2 of the ns=0 proj column block, scheduled into the
            otherwise starved first t=3 chunk; ci=3 lands in proj_piece."""

            def f():
                pp = ps_mm.tile([P, 512], F32, name=f"pp{co}", tag="mm")
                for ci in range(NPAIR - 1):
                    nc.tensor.matmul(
                        pp,
                        lhsT=pw_sb[:, ci, co * P : (co + 1) * P],
                        rhs=ot[ci][:, 0:512],
                        start=(ci == 0),
                        stop=(ci == NPAIR - 2),
                    )
                acc = sb.tile(
                    [P, 512], BF16, name=f"acc{co}", tag="acc", bufs=NCI
                )
                nc.vector.tensor_scalar(
                    acc, pp, bias_sb[:, co : co + 1], None,
                    op0=mybir.AluOpType.add,
                )
                acc_tiles[co] = acc

            return f

        def proj_piece(ns, co):
            def f():
                nsl = slice(ns * 512, (ns + 1) * 512)
                cosl = slice(co * P, (co + 1) * P)
                y_ps = ps_mm.tile([P, 512], F32, name=f"y{ns}_{co}", tag="mm")
                ci0 = NPAIR - 1 if ns == 0 else 0
                for ci in range(ci0, NPAIR):
                    nc.tensor.matmul(
                        y_ps,
                        lhsT=pw_sb[:, ci, cosl],
                        rhs=ot[ci][:, nsl],
                        start=(ci == ci0),
                        stop=(ci == NPAIR - 1),
                    )
                y_sb = sb.tile(
                    [P, 512], BF16, name=f"ysb{ns}_{co}", tag="y", bufs=3
                )
                if ns == 0:
                    # bias already rode in with the ci 0..2 partial
                    nc.vector.tensor_add(y_sb, y_ps, acc_tiles.pop(co))
                elif not (ns == NQC - 1 and co == NCI - 1):
                    # scalar engine has slack behind the exp stream; keeping
                    # DVE clear lets the boundary O copies start on time
                    nc.scalar.activation(
                        y_sb, y_ps,
                        mybir.ActivationFunctionType.Identity,
                        bias=bias_sb[:, co : co + 1],
                    )
                else:
                    # very last piece: DVE is idle, runs in parallel with
                    # the scalar-engine add of the previous piece
                    nc.vector.tensor_scalar(
                        y_sb, y_ps, bias_sb[:, co : co + 1], None,
                        op0=mybir.AluOpType.add,
                    )
                nc.sync.dma_start(out=yT_re[:, co, nsl], in_=y_sb)

            return f

        pending = deque()
        push_at = defaultdict(list)
        held = [None]  # deferred proj piece: boundary cushion for t=3 chunks

        def chunk_done(i, g):
            """Chunk i's last h0-PV just emitted at slot g (= 16(i+1)+1)."""
            t, qc = divmod(i, NQC)
            if held[0] is not None:
                push_at[g + 1].append(held[0])
                held[0] = None
            if t < NPAIR - 1:
                lo, cells = qk_lo_pair(t + 1, qc)
                push_at[g + 1].append(lo)
                push_at[g + 2].append(qk_hi(t + 1, qc, "q", cells))
                push_at[g + 2].append(qk_hi(t + 1, qc, "k", cells))
                push_at[g + 2].extend(norm_fillers(i))
            else:
                push_at[g + 2].extend(norm_fillers(i))
                last = NCI if qc == NQC - 1 else NCI - 1
                if qc == NQC - 2:
                    # hold two pieces back for the drain slots, where the
                    # PE would otherwise idle behind the last O-copy chain
                    last = NCI - 3
                    push_at[NSLOT].append(proj_piece(qc, NCI - 3))
                    push_at[NSLOT + 1].append(proj_piece(qc, NCI - 2))
                push_at[g + 2].extend(
                    proj_piece(qc, co) for co in range(last)
                )
                if qc < NQC - 1:
                    held[0] = proj_piece(qc, NCI - 1)
            if i == 3 * NQC - 1:  # (2,3) done: ns=0 partials fill (3,0)
                push_at[g + 2].extend(
                    partial_piece(co) for co in range(NCI)
                )
            if i == 0:  # late weight loads, clear of the startup x stream
                nc.sync.dma_start(out=wq23, in_=wq_re[:, :, 2 * P : 4 * P])
                nc.sync.dma_start(out=wk23, in_=wk_re[:, :, 2 * P : 4 * P])
            if i == 8:  # (t=2, qc=0) done: preload proj weights
                nc.sync.dma_start(out=pw_sb, in_=pw_re)

        def run_slot_tail(g):
            """PV emissions for slot g (h0 at lag 2, h1 at lag 3).  The
            PSUM->SBUF copies of finished O accumulators are emitted after
            BOTH PVs so a copy never blocks the other head's accumulation
            (whole-tile WAR)."""
            copies = []
            j0 = g - 2
            if 0 <= j0 < NSLOT:
                copies.append(emit_pv(j0, 0))
                if j0 % NKC == NKC - 1:
                    chunk_done(j0 // NKC, g)
            j1 = g - 3
            if 0 <= j1 < NSLOT:
                copies.append(emit_pv(j1, 1))
            for c in copies:
                if c is not None:
                    c()

        # ---- chunk-0 prologue, ordered to match DMA arrivals; dummy
        # matmuls plug the data-starved holes so the PE p-state stays hot ----
        alloc_qk(0)
        cell_q0 = emit_qk_piece(0, "q", 0, 256, c0=0, c1=4)
        emit_qk_piece(0, "q", 0, 256, c0=4, c1=NCI, ps_cell=cell_q0)
        emit_qk_piece(0, "k", 0, 256)
        emit_qk_piece(0, "q", 256, 256)
        emit_qk_piece(0, "k", 256, 256)
        emit_s(0)
        emit_s(1)
        emit_s(2)
        emit_s(3)
        s_next[0] = 4
        warm(3)
        cv0 = emit_v(0, c0=0, c1=4)
        cv1 = emit_v(1, c0=0, c1=4)
        emit_v(0, c0=4, c1=NCI, cell=cv0)
        emit_v(1, c0=4, c1=NCI, cell=cv1)

        # ---- chunk-0 slots (V pass + remaining qk(0) pieces folded in) ----
        for kc in range(NKC):
            if kc + 2 < NKC:
                emit_v(kc + 2)
            if kc in (0, 4, 8):
                nb = kc // 4 + 1
                emit_qk_piece(0, "k", nb * 512, 512)
                emit_qk_piece(0, "q", nb * 512, 512)
            emit_s_upto(kc)
            run_slot_tail(kc)

        # ---- steady-state slots ----
        for g in range(NKC, NSLOT + 3):
            for f in push_at.pop(g, ()):
                pending.append(f)
            if g < NSLOT:
                if pending:
                    pending.popleft()()
                emit_s_upto(g)
                run_slot_tail(g)
            else:
                # drain slots: PVs + O copies first so their chain starts,
                # then filler work covers the wait
                run_slot_tail(g)
                if pending:
                    pending.popleft()()

        # ---- tail drain ----
        for g in sorted(push_at):
            pending.extend(push_at[g])
        push_at.clear()
        while pending:
            pending.popleft()()

    nc.compile()
    return nc


def get_nc():
    if "nc" not in _CACHE:
        _CACHE["nc"] = _build()
    return _CACHE["nc"]


def make_in_maps(x, qkv_w, proj_w, proj_b):
    import ml_dtypes

    bf = ml_dtypes.bfloat16
    x = np.asarray(x, dtype=np.float32)
    qkv_w = np.asarray(qkv_w, dtype=np.float32)
    proj_w = np.asarray(proj_w, dtype=np.float32)
    proj_b = np.asarray(proj_b, dtype=np.float32)
    in_maps = []
    for d in range(8):
        b, g = d // 2, d % 2
        gs = slice(CH * g, CH * (g + 1))
        in_maps.append(
            {
                "xT": np.ascontiguousarray(x[b].T).astype(bf),
                "wq": np.ascontiguousarray(
                    qkv_w[0 * C :][gs.start : gs.stop].T
                ).astype(bf),
                "wk": np.ascontiguousarray(
                    qkv_w[1 * C :][gs.start : gs.stop].T
                ).astype(bf),
                "wv": np.ascontiguousarray(
                    qkv_w[2 * C :][gs.start : gs.stop].T
                ).astype(bf),
                "pw": np.ascontiguousarray(proj_w[:, gs].T).astype(bf),
                "bias": proj_b if g == 0 else np.zeros_like(proj_b),
            }
        )
    return in_maps


def kernel(x, qkv_w, proj_w, proj_b):
    global LAST_EXEC_TIME_NS
    from concourse import bass_utils

    nc = get_nc()
    in_maps = make_in_maps(x, qkv_w, proj_w, proj_b)
    res = bass_utils.run_bass_kernel_spmd(
        nc, in_maps, core_ids=list(range(8))
    )
    LAST_EXEC_TIME_NS = res.exec_time_ns
    out = np.empty((B, N, C), dtype=np.float32)
    for b in range(B):
        out[b] = (
            res.results[2 * b]["yT"].astype(np.float32)
            + res.results[2 * b + 1]["yT"].astype(np.float32)
        ).T
    return out


# revision 81
# speedup vs baseline: 1.0007x; 1.0007x over previous
"""Multi-head attention (B=4, N=2048, C=1024, H=16, D=64) on 8 TRN2 NeuronCores.

Sharding: data-parallel over batch (4) x tensor-parallel over heads (2 groups
of 8 heads).  Device d handles batch d//2 and head-group d%2.  Host sums the
two proj partials per batch and transposes back.

Kernel structure: one flat software pipeline over 256 (head-pair, q-chunk,
key-chunk) slots.  Slot g emits S(g) on the tensor engine + exp(g) on the
scalar engine; PV for head 0 runs at lag 2 and head 1 at lag 3 (the stagger
lets the PSUM->SBUF copy of the finished O accumulator overlap the next
chunk's first PV without double-buffering PSUM).  All remaining tensor-engine
work (V pass, q/k projections for later head pairs, softmax-denominator
reciprocal broadcast, output projection) drains from a slot-scheduled pending
queue so the PE stream never idles.  The S/exp stream runs up to ~16 slots
ahead of PV, banking exp results in a deep e-tile ring during the PE-heavy
first chunk so the scalar engine's exp throughput never paces the pipeline.
Matmul operands are bf16 (same PE cost in fp32-accumulate matmuls, half the
DMA/SBUF footprint); PSUM accumulation, softmax denominators and reciprocals
stay fp32.  A short warm-up of throwaway matmuls holds the PE busy from t=0
so the p-state ramp finishes before real work.
"""

import os
import sys

for _p in ("/opt/trn_rl_repo", "/root/.axon_site/_ro/trn_rl_repo"):
    if os.path.isdir(_p) and _p not in sys.path:
        sys.path.insert(0, _p)

import numpy as np

B, N, C = 4, 2048, 1024
H_LOC = 8  # heads per device
D = 64
CH = 512  # qkv channels per device (H_LOC * D)
P = 128
SCALE = 0.125  # D ** -0.5
NKC = N // P  # 16 key chunks
NQC = N // 512  # 4 query chunks of 512
NCI = C // P  # 8 c_in chunks
NPAIR = 4  # head pairs per device
NCHUNK = NPAIR * NQC  # 16
NSLOT = NCHUNK * NKC  # 256

_CACHE = {}
LAST_EXEC_TIME_NS = None


def _build():
    import concourse.bacc as bacc
    import concourse.mybir as mybir
    import concourse.tile as tile

    F32 = mybir.dt.float32
    F32R = mybir.dt.float32r
    BF16 = mybir.dt.bfloat16
    Exp = mybir.ActivationFunctionType.Exp

    nc = bacc.Bacc("TRN2", target_bir_lowering=False, debug=False)

    xT_d = nc.dram_tensor("xT", [C, N], BF16, kind="ExternalInput")
    wq_d = nc.dram_tensor("wq", [C, CH], BF16, kind="ExternalInput")
    wk_d = nc.dram_tensor("wk", [C, CH], BF16, kind="ExternalInput")
    wv_d = nc.dram_tensor("wv", [C, CH], BF16, kind="ExternalInput")
    pw_d = nc.dram_tensor("pw", [CH, C], BF16, kind="ExternalInput")
    bias_d = nc.dram_tensor("bias", [C], F32, kind="ExternalInput")
    yT_d = nc.dram_tensor("yT", [C, N], BF16, kind="ExternalOutput")

    xT_re = xT_d[:].rearrange("(c p) n -> p c n", p=P)
    wq_re = wq_d[:].rearrange("(c p) m -> p c m", p=P)
    wk_re = wk_d[:].rearrange("(c p) m -> p c m", p=P)
    wv_re = wv_d[:].rearrange("(c p) m -> p c m", p=P)
    pw_re = pw_d[:].rearrange("(c p) m -> p c m", p=P)
    bias_re = bias_d[:].rearrange("(a p) -> p a", p=P)
    yT_re = yT_d[:].rearrange("(a p) n -> p a n", p=P)

    from collections import defaultdict, deque
    from contextlib import ExitStack

    with tile.TileContext(nc) as tc, ExitStack() as st:
        ps_s = st.enter_context(tc.tile_pool(name="ps_s", bufs=2, space="PSUM"))
        ps_o = st.enter_context(tc.tile_pool(name="ps_o", bufs=1, space="PSUM"))
        ps_mm = st.enter_context(tc.tile_pool(name="ps_mm", bufs=2, space="PSUM"))
        sb = st.enter_context(tc.tile_pool(name="sb", bufs=1))

        # ---- persistent SBUF tiles ----
        xsb = sb.tile([P, NCI, N], BF16)
        wv_sb = sb.tile([P, NCI, CH], BF16)
        wq01 = sb.tile([P, NCI, 2 * P], BF16)
        wk01 = sb.tile([P, NCI, 2 * P], BF16)
        wq23 = sb.tile([P, NCI, 2 * P], BF16)
        wk23 = sb.tile([P, NCI, 2 * P], BF16)
        v_sb = sb.tile([P, NKC, H_LOC * 65], BF16)
        ones_src = sb.tile([P, NKC, H_LOC], F32)
        oc = sb.tile([1, D], F32R)
        ocH = sb.tile([65, D], F32R)  # row 64 = ones, same partition as denom
        dmy = sb.tile([1, 512], F32R)
        ot = [
            sb.tile([P, N], BF16, name=f"ot{t}", tag=f"ot{t}")
            for t in range(NPAIR)
        ]
        pw_sb = sb.tile([P, NPAIR, C], BF16)
        bias_sb = sb.tile([P, NCI], F32)

        # ---- constants (dummy-matmul inputs first so warm-up starts asap;
        # a narrow dmy slice lets the first dummies issue ~1us earlier) ----
        nc.vector.memset(oc.bitcast(F32), 1.0)
        nc.vector.memset(dmy.bitcast(F32)[:, 0:128], 1.0)
        nc.vector.memset(dmy.bitcast(F32)[:, 128:512], 1.0)
        nc.vector.memset(ocH.bitcast(F32), 1.0)
        nc.vector.memset(ones_src, 1.0)
        nc.vector.tensor_copy(
            v_sb.rearrange("p k (h e) -> p k h e", e=65)[:, :, :, 64:65],
            ones_src.rearrange("p k (h o) -> p k h o", o=1),
        )

        # ---- PE warm-up: hold the engine busy through the p-state ramp ----
        n_wrm = [0]

        def warm(k, w=512):
            for _ in range(k):
                wrm = ps_o.tile(
                    [D, w], F32, name=f"wrm{n_wrm[0]}", tag="o"
                )
                n_wrm[0] += 1
                nc.tensor.matmul(
                    wrm, lhsT=oc, rhs=dmy[:, 0:w], start=True, stop=True
                )

        warm(4, w=128)
        warm(14)

        # ---- startup DMAs (weights on sync queue, x stream on gpsimd) ----
        # first weight/x loads split in c-halves: the first half of the
        # first q piece can start ~1.5us earlier
        nc.sync.dma_start(out=wq01[:, 0:4, :], in_=wq_re[:, 0:4, 0 : 2 * P])
        nc.gpsimd.dma_start(out=xsb[:, 0:4, 0:256], in_=xT_re[:, 0:4, 0:256])
        nc.sync.dma_start(out=wq01[:, 4:8, :], in_=wq_re[:, 4:8, 0 : 2 * P])
        nc.gpsimd.dma_start(out=xsb[:, 4:8, 0:256], in_=xT_re[:, 4:8, 0:256])
        # tiny shim: delays wk01's HWDGE slot just enough that the first x
        # slice wins the DMA-engine grant race (and loads bias early)
        nc.sync.dma_start(out=bias_sb, in_=bias_re)
        nc.sync.dma_start(out=wv_sb[:, 0:4, :], in_=wv_re[:, 0:4, :])
        nc.gpsimd.dma_start(out=xsb[:, :, 256:512], in_=xT_re[:, :, 256:512])
        nc.sync.dma_start(out=wk01, in_=wk_re[:, :, 0 : 2 * P])
        nc.sync.dma_start(out=wv_sb[:, 4:8, :], in_=wv_re[:, 4:8, :])
        for nb in range(1, NQC):
            sl = slice(nb * 512, (nb + 1) * 512)
            nc.gpsimd.dma_start(out=xsb[:, :, sl], in_=xT_re[:, :, sl])

        # ---- helpers ----
        qk_tiles = {}  # t -> (qT, kT)

        def alloc_qk(t):
            qT_t = sb.tile([P, N], BF16, name=f"qT{t}", tag="qT", bufs=2)
            kT_t = sb.tile([P, N], BF16, name=f"kT{t}", tag="kT", bufs=2)
            qk_tiles[t] = (qT_t, kT_t)

        def w_src(t, kind):
            w = (wq01 if kind == "q" else wk01) if t < 2 else (
                wq23 if kind == "q" else wk23
            )
            csl = slice((t % 2) * P, (t % 2) * P + P)
            return w, csl

        piece_ready = set()  # (t, kind, nb) -> q/k projection piece emitted

        def emit_qk_piece(t, kind, col0, w, c0=0, c1=NCI, ps_cell=None):
            """Partial q/k projection: rows of qT/kT[t], cols [col0, col0+w)."""
            w_sb, csl = w_src(t, kind)
            dst = qk_tiles[t][0 if kind == "q" else 1]
            if ps_cell is None:
                ps_cell = {}
            if c0 == 0:
                ps_cell["ps"] = ps_mm.tile(
                    [P, w], F32, name=f"qk{t}{kind}{col0}", tag="mm"
                )
            ps = ps_cell["ps"]
            nsl = slice(col0, col0 + w)
            for c in range(c0, c1):
                nc.tensor.matmul(
                    ps,
                    lhsT=w_sb[:, c, csl],
                    rhs=xsb[:, c, nsl],
                    start=(c == 0),
                    stop=(c == NCI - 1),
                )
            if c1 == NCI:
                nc.vector.tensor_copy(dst[:, nsl], ps)
                if (col0 + w) % 512 == 0:
                    piece_ready.add((t, kind, (col0 + w - 1) // 512))
            return ps_cell

        def emit_v(kc, c0=0, c1=NCI, cell=None):
            if cell is None:
                cell = {}
            if c0 == 0:
                cell["ps"] = ps_mm.tile(
                    [P, CH], F32, name=f"v{kc}", tag="mm"
                )
            v_ps = cell["ps"]
            ksl = slice(kc * P, (kc + 1) * P)
            for c in range(c0, c1):
                nc.tensor.matmul(
                    v_ps,
                    lhsT=xsb[:, c, ksl],
                    rhs=wv_sb[:, c, :],
                    start=(c == 0),
                    stop=(c == NCI - 1),
                )
            if c1 < NCI:
                return cell
            nc.vector.tensor_copy(
                v_sb.rearrange("p k (h e) -> p k h e", e=65)[:, kc, :, 0:64],
                v_ps.rearrange("p (h e) -> p h e", e=64),
            )

        e_tiles = {}
        E_BUFS = 20
        s_next = [0]

        def emit_s(g):
            i, kc = divmod(g, NKC)
            t, qc = divmod(i, NQC)
            qT_t, kT_t = qk_tiles[t]
            ksl = slice(kc * P, (kc + 1) * P)
            qsl = slice(qc * 512, (qc + 1) * 512)
            s = ps_s.tile([P, 1024], F32, name=f"s{g}", tag="s")
            nc.tensor.matmul(
                s[:, 0:512], lhsT=kT_t[0:D, ksl], rhs=qT_t[0:D, qsl],
                start=True, stop=True,
            )
            nc.tensor.matmul(
                s[:, 512:1024], lhsT=kT_t[D:P, ksl], rhs=qT_t[D:P, qsl],
                start=True, stop=True,
            )
            e = sb.tile([P, 1024], BF16, name=f"e{g}", tag="e", bufs=E_BUFS)
            nc.scalar.activation(e, s, Exp, scale=SCALE)
            e_tiles[g] = e

        def s_deps_ready(j):
            i, kc = divmod(j, NKC)
            t, qc = divmod(i, NQC)
            return (t, "q", qc) in piece_ready and (
                t, "k", kc // 4
            ) in piece_ready

        def emit_s_upto(g, budget=2):
            """Emit S/exp as far ahead of the PV stream as the e-tile ring
            and the q/k piece availability allow -- banking exps during
            PE-heavy phases so the scalar engine never paces the pipeline."""
            while (
                budget > 0
                and s_next[0] < NSLOT
                and s_next[0] <= g + E_BUFS - 4
                and s_deps_ready(s_next[0])
            ):
                emit_s(s_next[0])
                s_next[0] += 1
                budget -= 1

        o_ps_tiles = {}
        osb_tiles = {}

        def emit_pv(j, hh):
            i, pv = divmod(j, NKC)
            t = i // NQC
            h = 2 * t + hh
            if pv == 0 and hh == 0:
                o_ps_tiles[i] = ps_o.tile(
                    [65, 1024], F32, name=f"o{i}", tag="o"
                )
            o_ps = o_ps_tiles[i]
            e = e_tiles[j]
            nc.tensor.matmul(
                o_ps[:, 512 * hh : 512 * hh + 512],
                lhsT=v_sb[:, pv, 65 * h : 65 * h + 65],
                rhs=e[:, 512 * hh : 512 * hh + 512],
                start=(pv == 0),
                stop=(pv == NKC - 1),
            )
            if hh == 1:
                del e_tiles[j]
            if pv == NKC - 1:

                def copy_out():
                    osb = sb.tile(
                        [65, 512], F32R, name=f"osb{i}_{hh}",
                        tag=f"osb{hh}", bufs=2,
                    )
                    if i == NCHUNK - 1 and hh == 1:
                        # tail: scalar engine is idle; parallel O copies
                        nc.scalar.copy(
                            osb, o_ps[:, 512 * hh : 512 * hh + 512]
                        )
                    else:
                        nc.vector.tensor_copy(
                            osb, o_ps[:, 512 * hh : 512 * hh + 512]
                        )
                    osb_tiles[(i, hh)] = osb

                return copy_out
            return None

        def norm_fillers(i):
            t, qc = divmod(i, NQC)
            qsl = slice(qc * 512, (qc + 1) * 512)
            out = []
            rb_cell = {}

            def recip_rb(hh):
                def f():
                    osb = osb_tiles[(i, hh)]
                    with nc.allow_low_precision(
                        reason="f32r is bit-identical to f32 here"
                    ):
                        nc.vector.reciprocal(osb[64:65, :], osb[64:65, :])
                    rb = ps_mm.tile(
                        [D, 512], F32, name=f"rb{i}_{hh}", tag="mm"
                    )
                    nc.tensor.matmul(
                        rb, lhsT=ocH[64:65, :], rhs=osb[64:65, :],
                        start=True, stop=True,
                    )
                    rb_cell[hh] = rb

                return f

            def mult(hh):
                def f():
                    osb = osb_tiles.pop((i, hh))
                    nc.vector.tensor_mul(
                        ot[t][64 * hh : 64 * hh + 64, qsl],
                        osb[0:64, :],
                        rb_cell.pop(hh),
                    )

                return f

            return [recip_rb(0), recip_rb(1), mult(0), mult(1)]

        def qk_lo_pair(t, nb):
            cells = {}

            def f():
                if t not in qk_tiles:
                    alloc_qk(t)
                for kind in ("q", "k"):
                    cells[kind] = emit_qk_piece(
                        t, kind, nb * 512, 512, c0=0, c1=4
                    )

            return f, cells

        def qk_hi(t, nb, kind, cells):
            def f():
                emit_qk_piece(
                    t, kind, nb * 512, 512, c0=4, c1=NCI,
                    ps_cell=cells[kind],
                )

            return f

        acc_tiles = {}

        def partial_piece(co):
            """ci 0..2 of the ns=0 proj column block, scheduled into the
            otherwise starved first t=3 chunk; ci=3 lands in proj_piece."""

            def f():
                pp = ps_mm.tile([P, 512], F32, name=f"pp{co}", tag="mm")
                for ci in range(NPAIR - 1):
                    nc.tensor.matmul(
                        pp,
                        lhsT=pw_sb[:, ci, co * P : (co + 1) * P],
                        rhs=ot[ci][:, 0:512],
                        start=(ci == 0),
                        stop=(ci == NPAIR - 2),
                    )
                acc = sb.tile(
                    [P, 512], BF16, name=f"acc{co}", tag="acc", bufs=NCI
                )
                nc.vector.tensor_scalar(
                    acc, pp, bias_sb[:, co : co + 1], None,
                    op0=mybir.AluOpType.add,
                )
                acc_tiles[co] = acc

            return f

        def proj_piece(ns, co):
            def f():
                nsl = slice(ns * 512, (ns + 1) * 512)
                cosl = slice(co * P, (co + 1) * P)
                y_ps = ps_mm.tile([P, 512], F32, name=f"y{ns}_{co}", tag="mm")
                ci0 = NPAIR - 1 if ns == 0 else 0
                for ci in range(ci0, NPAIR):
                    nc.tensor.matmul(
                        y_ps,
                        lhsT=pw_sb[:, ci, cosl],
                        rhs=ot[ci][:, nsl],
                        start=(ci == ci0),
                        stop=(ci == NPAIR - 1),
                    )
                y_sb = sb.tile(
                    [P, 512], BF16, name=f"ysb{ns}_{co}", tag="y", bufs=3
                )
                if ns == 0:
                    # bias already rode in with the ci 0..2 partial
                    nc.vector.tensor_add(y_sb, y_ps, acc_tiles.pop(co))
                elif not (ns == NQC - 1 and co == NCI - 1):
                    # scalar engine has slack behind the exp stream; keeping
                    # DVE clear lets the boundary O copies start on time
                    nc.scalar.activation(
                        y_sb, y_ps,
                        mybir.ActivationFunctionType.Identity,
                        bias=bias_sb[:, co : co + 1],
                    )
                else:
                    # very last piece: DVE is idle, runs in parallel with
                    # the scalar-engine add of the previous piece
                    nc.vector.tensor_scalar(
                        y_sb, y_ps, bias_sb[:, co : co + 1], None,
                        op0=mybir.AluOpType.add,
                    )
                nc.sync.dma_start(out=yT_re[:, co, nsl], in_=y_sb)

            return f

        pending = deque()
        push_at = defaultdict(list)
        held = [None]  # deferred proj piece: boundary cushion for t=3 chunks

        def chunk_done(i, g):
            """Chunk i's last h0-PV just emitted at slot g (= 16(i+1)+1)."""
            t, qc = divmod(i, NQC)
            if held[0] is not None:
                push_at[g + 1].append(held[0])
                held[0] = None
            if t < NPAIR - 1:
                lo, cells = qk_lo_pair(t + 1, qc)
                push_at[g + 1].append(lo)
                push_at[g + 2].append(qk_hi(t + 1, qc, "q", cells))
                push_at[g + 2].append(qk_hi(t + 1, qc, "k", cells))
                push_at[g + 2].extend(norm_fillers(i))
            else:
                push_at[g + 2].extend(norm_fillers(i))
                last = NCI if qc == NQC - 1 else NCI - 1
                if qc == NQC - 2:
                    # hold two pieces back for the drain slots, where the
                    # PE would otherwise idle behind the last O-copy chain
                    last = NCI - 3
                    push_at[NSLOT].append(proj_piece(qc, NCI - 3))
                    push_at[NSLOT + 1].append(proj_piece(qc, NCI - 2))
                push_at[g + 2].extend(
                    proj_piece(qc, co) for co in range(last)
                )
                if qc < NQC - 1:
                    held[0] = proj_piece(qc, NCI - 1)
            if i == 3 * NQC - 1:  # (2,3) done: ns=0 partials fill (3,0)
                push_at[g + 2].extend(
                    partial_piece(co) for co in range(NCI)
                )
            if i == 0:  # late weight loads, clear of the startup x stream
                nc.sync.dma_start(out=wq23, in_=wq_re[:, :, 2 * P : 4 * P])
                nc.sync.dma_start(out=wk23, in_=wk_re[:, :, 2 * P : 4 * P])
            if i == 8:  # (t=2, qc=0) done: preload proj weights
                nc.sync.dma_start(out=pw_sb, in_=pw_re)

        def run_slot_tail(g):
            """PV emissions for slot g (h0 at lag 2, h1 at lag 3).  The
            PSUM->SBUF copies of finished O accumulators are emitted after
            BOTH PVs so a copy never blocks the other head's accumulation
            (whole-tile WAR)."""
            copies = []
            j0 = g - 2
            if 0 <= j0 < NSLOT:
                copies.append(emit_pv(j0, 0))
                if j0 % NKC == NKC - 1:
                    chunk_done(j0 // NKC, g)
            j1 = g - 3
            if 0 <= j1 < NSLOT:
                copies.append(emit_pv(j1, 1))
            for c in copies:
                if c is not None:
                    c()

        # ---- chunk-0 prologue, ordered to match DMA arrivals; dummy
        # matmuls plug the data-starved holes so the PE p-state stays hot ----
        alloc_qk(0)
        cell_q0 = emit_qk_piece(0, "q", 0, 256, c0=0, c1=4)
        emit_qk_piece(0, "q", 0, 256, c0=4, c1=NCI, ps_cell=cell_q0)
        emit_qk_piece(0, "k", 0, 256)
        emit_qk_piece(0, "q", 256, 256)
        emit_qk_piece(0, "k", 256, 256)
        emit_s(0)
        emit_s(1)
        emit_s(2)
        emit_s(3)
        s_next[0] = 4
        warm(3)
        cv0 = emit_v(0, c0=0, c1=4)
        cv1 = emit_v(1, c0=0, c1=4)
        emit_v(0, c0=4, c1=NCI, cell=cv0)
        emit_v(1, c0=4, c1=NCI, cell=cv1)

        # ---- chunk-0 slots (V pass + remaining qk(0) pieces folded in) ----
        for kc in range(NKC):
            if kc + 2 < NKC:
                emit_v(kc + 2)
            if kc in (0, 4, 8):
                nb = kc // 4 + 1
                emit_qk_piece(0, "k", nb * 512, 512)
                emit_qk_piece(0, "q", nb * 512, 512)
            emit_s_upto(kc)
            run_slot_tail(kc)

        # ---- steady-state slots ----
        for g in range(NKC, NSLOT + 3):
            for f in push_at.pop(g, ()):
                pending.append(f)
            if g < NSLOT:
                if pending:
                    pending.popleft()()
                emit_s_upto(g)
                run_slot_tail(g)
            else:
                # drain slots: PVs + O copies first so their chain starts,
                # then filler work covers the wait
                run_slot_tail(g)
                if pending:
                    pending.popleft()()

        # ---- tail drain ----
        for g in sorted(push_at):
            pending.extend(push_at[g])
        push_at.clear()
        while pending:
            pending.popleft()()

    nc.compile()
    return nc


def get_nc():
    if "nc" not in _CACHE:
        _CACHE["nc"] = _build()
    return _CACHE["nc"]


def make_in_maps(x, qkv_w, proj_w, proj_b):
    import ml_dtypes

    bf = ml_dtypes.bfloat16
    x = np.asarray(x, dtype=np.float32)
    qkv_w = np.asarray(qkv_w, dtype=np.float32)
    proj_w = np.asarray(proj_w, dtype=np.float32)
    proj_b = np.asarray(proj_b, dtype=np.float32)
    in_maps = []
    for d in range(8):
        b, g = d // 2, d % 2
        gs = slice(CH * g, CH * (g + 1))
        in_maps.append(
            {
                "xT": np.ascontiguousarray(x[b].T).astype(bf),
                "wq": np.ascontiguousarray(
                    qkv_w[0 * C :][gs.start : gs.stop].T
                ).astype(bf),
                "wk": np.ascontiguousarray(
                    qkv_w[1 * C :][gs.start : gs.stop].T
                ).astype(bf),
                "wv": np.ascontiguousarray(
                    qkv_w[2 * C :][gs.start : gs.stop].T
                ).astype(bf),
                "pw": np.ascontiguousarray(proj_w[:, gs].T).astype(bf),
                "bias": proj_b if g == 0 else np.zeros_like(proj_b),
            }
        )
    return in_maps


def kernel(x, qkv_w, proj_w, proj_b):
    global LAST_EXEC_TIME_NS
    from concourse import bass_utils

    nc = get_nc()
    in_maps = make_in_maps(x, qkv_w, proj_w, proj_b)
    res = bass_utils.run_bass_kernel_spmd(
        nc, in_maps, core_ids=list(range(8))
    )
    LAST_EXEC_TIME_NS = res.exec_time_ns
    out = np.empty((B, N, C), dtype=np.float32)
    for b in range(B):
        out[b] = (
            res.results[2 * b]["yT"].astype(np.float32)
            + res.results[2 * b + 1]["yT"].astype(np.float32)
        ).T
    return out
